# revision 13
# baseline (speedup 1.0000x reference)
"""Trainium2 Bass kernel for the zonotope/interval AbstractMaxpool2D transformer.

Strategy: data-parallel over channels. Each of the 8 NeuronCores gets 8 of the
64 channels with the FULL symbol axis S resident per h-half group, so the
whole computation (interval reduction, window argmax/masks, gather, output
interval) is core-local — no collectives.

Per-core layout: SBUF partitions = (c8, ho16); each h-half of the image is one
"group".  x is streamed in s-chunks of 16 rows; the |eps| reduction runs as
ACT-abs + DVE-add while chunks arrive; the window masks are tiny [128,32]
ops; the gather is a one-hot multiply + 3 predicated copies; x_min/x_max come
from the identity  out_abs = exact ? eps_w[i*] : box_noise  (no second big
reduction needed).
"""

import numpy as np

S, C, H, W = 258, 64, 64, 64
NCORES = 8
CL = C // NCORES            # 8 local channels per core
HO, WO = H // 2, W // 2     # 32, 32
HOG = HO // 2               # 16 ho rows per group (h-half)
SCH = 16                    # s rows per chunk
NFULL = S // SCH            # 16 full chunks
TAIL = S - NFULL * SCH      # 2
CHUNKS = [(i * SCH, SCH) for i in range(NFULL)] + [(NFULL * SCH, TAIL)]
BIG = 1.0e38
WOP = WO + 1                 # pad so [p, s, wo] APs stay 3-D (sim view quirk)

_CACHE = {}


def _emit(ctx, tc, nc, mybir, bass, x_d, xt_d, y_d, ymin_d, ymax_d, ytrue_d):
    f32 = mybir.dt.float32
    Alu = mybir.AluOpType
    Act = mybir.ActivationFunctionType
    v = nc.vector
    sc = nc.scalar

    xin = ctx.enter_context(tc.tile_pool(name="xin", bufs=18))
    absp = ctx.enter_context(tc.tile_pool(name="absp", bufs=2))
    accp = ctx.enter_context(tc.tile_pool(name="accp", bufs=1))
    maskp = ctx.enter_context(tc.tile_pool(name="maskp", bufs=2))
    mrepp = ctx.enter_context(tc.tile_pool(name="mrepp", bufs=1))
    outp = ctx.enter_context(tc.tile_pool(name="outp", bufs=4))
    smallp = ctx.enter_context(tc.tile_pool(name="smallp", bufs=2))

    def win(ap3, j):
        # ap3: [128, 2, 64] -> window-element slice [128, 32]
        dh, dw = j // 2, j % 2
        return ap3[:, dh, dw::2]

    # ---------------- x_true maxpool (tiny, independent) ----------------
    for q in range(2):  # channel quads (4 channels x 32 ho = 128 partitions)
        # (c ho) merges cleanly here: full-height rows are contiguous per c
        xt_src = xt_d[4 * q:4 * q + 4].rearrange("c (ho dh) w -> (c ho) dh w", dh=2)
        t_e = smallp.tile([128, W], f32, name=f"xt_e{q}")
        t_o = smallp.tile([128, W], f32, name=f"xt_o{q}")
        nc.sync.dma_start(t_e[:], xt_src[:, 0, :])
        nc.sync.dma_start(t_o[:], xt_src[:, 1, :])
        t_m = smallp.tile([128, W], f32, name=f"xt_m{q}")
        v.tensor_tensor(t_m[:], t_e[:], t_o[:], op=Alu.max)
        t_p = smallp.tile([128, WO], f32, name=f"xt_p{q}")
        v.tensor_tensor(t_p[:], t_m[:, 0::2], t_m[:, 1::2], op=Alu.max)
        nc.sync.dma_start(
            ytrue_d[4 * q:4 * q + 4].rearrange("c ho wo -> (c ho) wo"), t_p[:])

    # ---------------- main pipeline: two h-half groups ----------------
    for g in range(2):
        hob = g * HOG  # first ho row of this group

        # ---- stream in + |eps| reduction ----
        xts = []
        acc = accp.tile([128, SCH, 2, W], f32, name="acc")
        v.memset(acc[:, 0:1], 0.0)
        for ci, (s0, L) in enumerate(CHUNKS):
            xt = xin.tile([128, SCH, 2, W], f32, name="xt", tag="xt")
            xts.append(xt)
            for si in range(L):
                src = x_d[s0 + si, :, 2 * hob:2 * hob + 2 * HOG, :].rearrange(
                    "c (ho dh) w -> c ho (dh w)", dh=2)
                eng = nc.sync if si % 2 == 0 else nc.scalar
                eng.dma_start(xt[:, si].rearrange("p dh w -> p (dh w)"), src)
            if ci == 0:
                # write |x| for s=1..15 straight into the accumulator
                sc.activation(acc[:, 1:L], xt[:, 1:L], Act.Abs)
            else:
                ab = absp.tile([128, SCH, 2, W], f32, name="ab", tag="ab")
                sc.activation(ab[:, :L], xt[:, :L], Act.Abs)
                v.tensor_add(acc[:, :L], acc[:, :L], ab[:, :L])

        # ---- fold 16 s-phases -> eps_w [128, 2, 64] ----
        v.tensor_add(acc[:, 0:8], acc[:, 0:8], acc[:, 8:16])
        v.tensor_add(acc[:, 0:4], acc[:, 0:4], acc[:, 4:8])
        v.tensor_add(acc[:, 0:2], acc[:, 0:2], acc[:, 2:4])
        eps_w = maskp.tile([128, 2, W], f32, name="eps_w")
        v.tensor_add(eps_w[:], acc[:, 0], acc[:, 1])

        # ---- window masks (all [128, 32]) ----
        x0 = xts[0][:, 0]  # center row, [128, 2, 64]
        lo_t = maskp.tile([128, 2, W], f32, name="lo_t")
        hi_t = maskp.tile([128, 2, W], f32, name="hi_t")
        v.tensor_sub(lo_t[:], x0, eps_w[:])
        v.tensor_add(hi_t[:], x0, eps_w[:])

        m01 = maskp.tile([128, WO], f32, name="m01")
        m23 = maskp.tile([128, WO], f32, name="m23")
        l_star = maskp.tile([128, WO], f32, name="l_star")
        v.tensor_tensor(m01[:], win(lo_t, 0), win(lo_t, 1), op=Alu.max)
        v.tensor_tensor(m23[:], win(lo_t, 2), win(lo_t, 3), op=Alu.max)
        v.tensor_tensor(l_star[:], m01[:], m23[:], op=Alu.max)

        u01 = maskp.tile([128, WO], f32, name="u01")
        u23 = maskp.tile([128, WO], f32, name="u23")
        u_max = maskp.tile([128, WO], f32, name="u_max")
        v.tensor_tensor(u01[:], win(hi_t, 0), win(hi_t, 1), op=Alu.max)
        v.tensor_tensor(u23[:], win(hi_t, 2), win(hi_t, 3), op=Alu.max)
        v.tensor_tensor(u_max[:], u01[:], u23[:], op=Alu.max)

        # one-hot of argmax(lw), first-index tie-break
        oh = [maskp.tile([128, WO], f32, name=f"oh{j}") for j in range(4)]
        rem = [maskp.tile([128, WO], f32, name=f"rem{j}") for j in range(3)]
        v.tensor_tensor(oh[0][:], win(lo_t, 0), l_star[:], op=Alu.is_equal)
        v.tensor_scalar(rem[0][:], oh[0][:], -1.0, 1.0, op0=Alu.mult, op1=Alu.add)
        cmp = maskp.tile([128, WO], f32, name="cmp")
        for j in (1, 2, 3):
            v.tensor_tensor(cmp[:], win(lo_t, j), l_star[:], op=Alu.is_equal)
            v.tensor_mul(oh[j][:], cmp[:], rem[j - 1][:])
            if j < 3:
                v.tensor_sub(rem[j][:], rem[j - 1][:], oh[j][:])

        # max upper bound among non-argmax elements
        uo = [maskp.tile([128, WO], f32, name=f"uo{j}") for j in range(4)]
        for j in range(4):
            v.scalar_tensor_tensor(uo[j][:], oh[j][:], -BIG, win(hi_t, j),
                                   op0=Alu.mult, op1=Alu.add)
        v.tensor_tensor(uo[0][:], uo[0][:], uo[1][:], op=Alu.max)
        v.tensor_tensor(uo[2][:], uo[2][:], uo[3][:], op=Alu.max)
        u_other = maskp.tile([128, WO], f32, name="u_other")
        v.tensor_tensor(u_other[:], uo[0][:], uo[2][:], op=Alu.max)

        exact = maskp.tile([128, WO], f32, name="exact")
        v.tensor_tensor(exact[:], l_star[:], u_other[:], op=Alu.is_ge)
        notex = maskp.tile([128, WO], f32, name="notex")
        v.tensor_scalar(notex[:], exact[:], -1.0, 1.0, op0=Alu.mult, op1=Alu.add)

        ohx = [maskp.tile([128, WO], f32, name=f"ohx{j}") for j in range(4)]
        for j in range(4):
            v.tensor_mul(ohx[j][:], oh[j][:], exact[:])
        i32 = mybir.dt.int32
        ohxi = [None] + [maskp.tile([128, WO], i32, name=f"ohxi{j}") for j in (1, 2, 3)]
        for j in (1, 2, 3):
            v.tensor_copy(ohxi[j][:], ohx[j][:])

        bsum = maskp.tile([128, WO], f32, name="bsum")
        bdif = maskp.tile([128, WO], f32, name="bdif")
        bc_term = maskp.tile([128, WO], f32, name="bc_term")
        bn_term = maskp.tile([128, WO], f32, name="bn_term")
        v.tensor_add(bsum[:], l_star[:], u_max[:])
        v.scalar_tensor_tensor(bc_term[:], bsum[:], 0.5, notex[:],
                               op0=Alu.mult, op1=Alu.mult)
        v.tensor_sub(bdif[:], u_max[:], l_star[:])
        v.scalar_tensor_tensor(bn_term[:], bdif[:], 0.5, notex[:],
                               op0=Alu.mult, op1=Alu.mult)

        # replicate select masks across the 16 s-phases of a chunk
        mrep = []
        for j in range(4):
            mdt = f32 if j == 0 else mybir.dt.int32
            msrc = ohx[j] if j == 0 else ohxi[j]
            mr = mrepp.tile([128, SCH, WOP], mdt, name=f"mrep{j}", tag=f"mrep{j}")
            v.tensor_copy(mr[:, 0:1, :WO], msrc[:].unsqueeze(1))
            v.tensor_copy(mr[:, 1:2, :WO], mr[:, 0:1, :WO])
            v.tensor_copy(mr[:, 2:4, :WO], mr[:, 0:2, :WO])
            v.tensor_copy(mr[:, 4:8, :WO], mr[:, 0:4, :WO])
            v.tensor_copy(mr[:, 8:16, :WO], mr[:, 0:8, :WO])
            mrep.append(mr)

        # ---- gather/select + output ----
        oc = None
        for ci, (s0, L) in enumerate(CHUNKS):
            xt = xts[ci]
            ot = outp.tile([128, SCH, WOP], f32, name="ot", tag="ot")

            def xw(j):
                dh, dw = j // 2, j % 2
                return xt[:, :L, dh, dw::2]

            v.tensor_mul(ot[:, :L, :WO], xw(0), mrep[0][:, :L, :WO])
            for j in (1, 2, 3):
                v.copy_predicated(ot[:, :L, :WO], mrep[j][:, :L, :WO], xw(j))

            if ci == 0:
                # center row: + (1-exact) * box_center
                v.tensor_add(ot[:, 0, :WO], ot[:, 0, :WO], bc_term[:])
                oc = maskp.tile([128, WO], f32, name="oc")
                v.tensor_copy(oc[:], ot[:, 0, :WO])
            if ci == len(CHUNKS) - 1:
                # noise row: abs, + (1-exact) * box_noise
                sc.activation(ot[:, L - 1, :WO], ot[:, L - 1, :WO], Act.Abs)
                v.tensor_add(ot[:, L - 1, :WO], ot[:, L - 1, :WO], bn_term[:])

            for si in range(L):
                dst = y_d[s0 + si, :, hob:hob + HOG, :]
                eng = nc.sync if si % 2 == 1 else nc.scalar
                eng.dma_start(dst, ot[:, si, :WO])

        # ---- x_min / x_max ----
        es = maskp.tile([128, WO], f32, name="es")
        v.tensor_mul(es[:], win(eps_w, 0), ohx[0][:])
        for j in (1, 2, 3):
            v.copy_predicated(es[:], ohxi[j][:], win(eps_w, j))
        oabs = maskp.tile([128, WO], f32, name="oabs")
        v.tensor_add(oabs[:], es[:], bn_term[:])
        xmin_t = maskp.tile([128, WO], f32, name="xmin_t")
        xmax_t = maskp.tile([128, WO], f32, name="xmax_t")
        v.tensor_sub(xmin_t[:], oc[:], oabs[:])
        v.tensor_add(xmax_t[:], oc[:], oabs[:])
        nc.sync.dma_start(ymin_d[:, hob:hob + HOG, :], xmin_t[:])
        nc.sync.dma_start(ymax_d[:, hob:hob + HOG, :], xmax_t[:])


def _build(repeat=1):
    from contextlib import ExitStack
    import concourse.bass as bass
    import concourse.tile as tile
    from concourse import bacc, mybir

    nc = bacc.Bacc("TRN2", target_bir_lowering=False, debug=False,
                   enable_asserts=False, num_devices=NCORES)
    f32 = mybir.dt.float32
    x_d = nc.dram_tensor("x", (S, CL, H, W), f32, kind="ExternalInput").ap()
    xt_d = nc.dram_tensor("x_true", (CL, H, W), f32, kind="ExternalInput").ap()
    y_d = nc.dram_tensor("y", (S, CL, HO, WO), f32, kind="ExternalOutput").ap()
    ymin_d = nc.dram_tensor("y_min", (CL, HO, WO), f32, kind="ExternalOutput").ap()
    ymax_d = nc.dram_tensor("y_max", (CL, HO, WO), f32, kind="ExternalOutput").ap()
    ytrue_d = nc.dram_tensor("y_true", (CL, HO, WO), f32, kind="ExternalOutput").ap()

    with tile.TileContext(nc) as tc:
        from contextlib import ExitStack as _ES
        for _ in range(repeat):
            with _ES() as ctx:
                _emit(ctx, tc, nc, mybir, bass, x_d, xt_d, y_d, ymin_d, ymax_d,
                      ytrue_d)
    nc.compile()
    return nc


def _get_nc():
    if "nc" not in _CACHE:
        _CACHE["nc"] = _build()
    return _CACHE["nc"]


def kernel(x, x_true, kernel_size, stride, trace=False):
    from concourse.bass_utils import run_bass_kernel_spmd

    x = np.asarray(x, dtype=np.float32)
    x_true = np.asarray(x_true, dtype=np.float32)
    assert int(kernel_size) == 2 and int(stride) == 2
    assert x.shape == (S, C, H, W) and x_true.shape == (C, H, W)

    nc = _get_nc()
    in_maps = [
        {
            "x": np.ascontiguousarray(x[:, m * CL:(m + 1) * CL]),
            "x_true": np.ascontiguousarray(x_true[m * CL:(m + 1) * CL]),
        }
        for m in range(NCORES)
    ]
    res = run_bass_kernel_spmd(nc, in_maps, core_ids=list(range(NCORES)),
                               trace=trace)
    if trace:
        _CACHE["last_results"] = res
    x_out = np.concatenate([r["y"] for r in res.results], axis=1)
    x_min = np.concatenate([r["y_min"] for r in res.results], axis=0)
    x_max = np.concatenate([r["y_max"] for r in res.results], axis=0)
    x_true_out = np.concatenate([r["y_true"] for r in res.results], axis=0)
    return x_out, x_min, x_max, x_true_out


# revision 14
# speedup vs baseline: 1.3436x; 1.3436x over previous
"""Trainium2 Bass kernel for the zonotope/interval AbstractMaxpool2D transformer.

Strategy: data-parallel over channels. Each of the 8 NeuronCores gets 8 of the
64 channels with the FULL symbol axis S resident per h-half group, so the
whole computation (interval reduction, window argmax/masks, gather, output
interval) is core-local — no collectives.

Per-core layout: SBUF partitions = (c8, ho16); each h-half of the image is one
"group".  x is streamed in s-chunks of 16 rows; the |eps| reduction runs as
ACT-abs + DVE-add while chunks arrive; the window masks are tiny [128,32]
ops; the gather is a one-hot multiply + 3 predicated copies; x_min/x_max come
from the identity  out_abs = exact ? eps_w[i*] : box_noise  (no second big
reduction needed).
"""

import numpy as np

S, C, H, W = 258, 64, 64, 64
NCORES = 8
CL = C // NCORES            # 8 local channels per core
HO, WO = H // 2, W // 2     # 32, 32
HOG = HO // 2               # 16 ho rows per group (h-half)
SCH = 16                    # s rows per chunk
NFULL = S // SCH            # 16 full chunks
TAIL = S - NFULL * SCH      # 2
CHUNKS = [(i * SCH, SCH) for i in range(NFULL)] + [(NFULL * SCH, TAIL)]
BIG = 1.0e38
WOP = WO + 1                 # pad so [p, s, wo] APs stay 3-D (sim view quirk)

_CACHE = {}


def _emit(ctx, tc, nc, mybir, bass, x_d, xt_d, y_d, ymin_d, ymax_d, ytrue_d, mode='full'):
    f32 = mybir.dt.float32
    Alu = mybir.AluOpType
    Act = mybir.ActivationFunctionType
    v = nc.vector
    sc = nc.scalar

    xin = ctx.enter_context(tc.tile_pool(name="xin", bufs=18))
    absp = ctx.enter_context(tc.tile_pool(name="absp", bufs=2))
    accp = ctx.enter_context(tc.tile_pool(name="accp", bufs=1))
    maskp = ctx.enter_context(tc.tile_pool(name="maskp", bufs=2))
    mrepp = ctx.enter_context(tc.tile_pool(name="mrepp", bufs=1))
    outp = ctx.enter_context(tc.tile_pool(name="outp", bufs=4))
    smallp = ctx.enter_context(tc.tile_pool(name="smallp", bufs=2))

    def win(ap3, j):
        # ap3: [128, 2, 64] -> window-element slice [128, 32]
        dh, dw = j // 2, j % 2
        return ap3[:, dh, dw::2]

    # ---------------- x_true maxpool (tiny, independent) ----------------
    for q in range(2):  # channel quads (4 channels x 32 ho = 128 partitions)
        # (c ho) merges cleanly here: full-height rows are contiguous per c
        xt_src = xt_d[4 * q:4 * q + 4].rearrange("c (ho dh) w -> (c ho) dh w", dh=2)
        t_e = smallp.tile([128, W], f32, name=f"xt_e{q}")
        t_o = smallp.tile([128, W], f32, name=f"xt_o{q}")
        nc.sync.dma_start(t_e[:], xt_src[:, 0, :])
        nc.sync.dma_start(t_o[:], xt_src[:, 1, :])
        t_m = smallp.tile([128, W], f32, name=f"xt_m{q}")
        v.tensor_tensor(t_m[:], t_e[:], t_o[:], op=Alu.max)
        t_p = smallp.tile([128, WO], f32, name=f"xt_p{q}")
        v.tensor_tensor(t_p[:], t_m[:, 0::2], t_m[:, 1::2], op=Alu.max)
        nc.sync.dma_start(
            ytrue_d[4 * q:4 * q + 4].rearrange("c ho wo -> (c ho) wo"), t_p[:])

    # ---------------- main pipeline: two h-half groups ----------------
    for g in range(2):
        hob = g * HOG  # first ho row of this group

        # ---- stream in + |eps| reduction ----
        xts = []
        acc = accp.tile([128, SCH, 2, W], f32, name="acc")
        v.memset(acc[:, 0:1], 0.0)
        for ci, (s0, L) in enumerate(CHUNKS):
            xt = xin.tile([128, SCH, 2, W], f32, name="xt", tag="xt")
            xts.append(xt)
            for si in range(L):
                src = x_d[s0 + si, :, 2 * hob:2 * hob + 2 * HOG, :].rearrange(
                    "c (ho dh) w -> c ho (dh w)", dh=2)
                eng = nc.sync if si % 2 == 0 else nc.scalar
                if mode != 'compute':
                    eng.dma_start(xt[:, si].rearrange("p dh w -> p (dh w)"), src)
            if mode == 'dma':
                continue
            if ci == 0:
                # write |x| for s=1..15 straight into the accumulator
                sc.activation(acc[:, 1:L], xt[:, 1:L], Act.Abs)
            else:
                ab = absp.tile([128, SCH, 2, W], f32, name="ab", tag="ab")
                sc.activation(ab[:, :L], xt[:, :L], Act.Abs)
                v.tensor_add(acc[:, :L], acc[:, :L], ab[:, :L])

        if mode == 'dma':
            continue
        # ---- fold 16 s-phases -> eps_w [128, 2, 64] ----
        v.tensor_add(acc[:, 0:8], acc[:, 0:8], acc[:, 8:16])
        v.tensor_add(acc[:, 0:4], acc[:, 0:4], acc[:, 4:8])
        v.tensor_add(acc[:, 0:2], acc[:, 0:2], acc[:, 2:4])
        eps_w = maskp.tile([128, 2, W], f32, name="eps_w")
        v.tensor_add(eps_w[:], acc[:, 0], acc[:, 1])

        # ---- window masks (all [128, 32]) ----
        x0 = xts[0][:, 0]  # center row, [128, 2, 64]
        lo_t = maskp.tile([128, 2, W], f32, name="lo_t")
        hi_t = maskp.tile([128, 2, W], f32, name="hi_t")
        v.tensor_sub(lo_t[:], x0, eps_w[:])
        v.tensor_add(hi_t[:], x0, eps_w[:])

        m01 = maskp.tile([128, WO], f32, name="m01")
        m23 = maskp.tile([128, WO], f32, name="m23")
        l_star = maskp.tile([128, WO], f32, name="l_star")
        v.tensor_tensor(m01[:], win(lo_t, 0), win(lo_t, 1), op=Alu.max)
        v.tensor_tensor(m23[:], win(lo_t, 2), win(lo_t, 3), op=Alu.max)
        v.tensor_tensor(l_star[:], m01[:], m23[:], op=Alu.max)

        u01 = maskp.tile([128, WO], f32, name="u01")
        u23 = maskp.tile([128, WO], f32, name="u23")
        u_max = maskp.tile([128, WO], f32, name="u_max")
        v.tensor_tensor(u01[:], win(hi_t, 0), win(hi_t, 1), op=Alu.max)
        v.tensor_tensor(u23[:], win(hi_t, 2), win(hi_t, 3), op=Alu.max)
        v.tensor_tensor(u_max[:], u01[:], u23[:], op=Alu.max)

        # one-hot of argmax(lw), first-index tie-break
        oh = [maskp.tile([128, WO], f32, name=f"oh{j}") for j in range(4)]
        rem = [maskp.tile([128, WO], f32, name=f"rem{j}") for j in range(3)]
        v.tensor_tensor(oh[0][:], win(lo_t, 0), l_star[:], op=Alu.is_equal)
        v.tensor_scalar(rem[0][:], oh[0][:], -1.0, 1.0, op0=Alu.mult, op1=Alu.add)
        cmp = maskp.tile([128, WO], f32, name="cmp")
        for j in (1, 2, 3):
            v.tensor_tensor(cmp[:], win(lo_t, j), l_star[:], op=Alu.is_equal)
            v.tensor_mul(oh[j][:], cmp[:], rem[j - 1][:])
            if j < 3:
                v.tensor_sub(rem[j][:], rem[j - 1][:], oh[j][:])

        # max upper bound among non-argmax elements
        uo = [maskp.tile([128, WO], f32, name=f"uo{j}") for j in range(4)]
        for j in range(4):
            v.scalar_tensor_tensor(uo[j][:], oh[j][:], -BIG, win(hi_t, j),
                                   op0=Alu.mult, op1=Alu.add)
        v.tensor_tensor(uo[0][:], uo[0][:], uo[1][:], op=Alu.max)
        v.tensor_tensor(uo[2][:], uo[2][:], uo[3][:], op=Alu.max)
        u_other = maskp.tile([128, WO], f32, name="u_other")
        v.tensor_tensor(u_other[:], uo[0][:], uo[2][:], op=Alu.max)

        exact = maskp.tile([128, WO], f32, name="exact")
        v.tensor_tensor(exact[:], l_star[:], u_other[:], op=Alu.is_ge)
        notex = maskp.tile([128, WO], f32, name="notex")
        v.tensor_scalar(notex[:], exact[:], -1.0, 1.0, op0=Alu.mult, op1=Alu.add)

        ohx = [maskp.tile([128, WO], f32, name=f"ohx{j}") for j in range(4)]
        for j in range(4):
            v.tensor_mul(ohx[j][:], oh[j][:], exact[:])
        i32 = mybir.dt.int32
        ohxi = [None] + [maskp.tile([128, WO], i32, name=f"ohxi{j}") for j in (1, 2, 3)]
        for j in (1, 2, 3):
            v.tensor_copy(ohxi[j][:], ohx[j][:])

        bsum = maskp.tile([128, WO], f32, name="bsum")
        bdif = maskp.tile([128, WO], f32, name="bdif")
        bc_term = maskp.tile([128, WO], f32, name="bc_term")
        bn_term = maskp.tile([128, WO], f32, name="bn_term")
        v.tensor_add(bsum[:], l_star[:], u_max[:])
        v.scalar_tensor_tensor(bc_term[:], bsum[:], 0.5, notex[:],
                               op0=Alu.mult, op1=Alu.mult)
        v.tensor_sub(bdif[:], u_max[:], l_star[:])
        v.scalar_tensor_tensor(bn_term[:], bdif[:], 0.5, notex[:],
                               op0=Alu.mult, op1=Alu.mult)

        # replicate select masks across the 16 s-phases of a chunk
        mrep = []
        for j in range(4):
            mdt = f32 if j == 0 else mybir.dt.int32
            msrc = ohx[j] if j == 0 else ohxi[j]
            mr = mrepp.tile([128, SCH, WOP], mdt, name=f"mrep{j}", tag=f"mrep{j}")
            v.tensor_copy(mr[:, 0:1, :WO], msrc[:].unsqueeze(1))
            v.tensor_copy(mr[:, 1:2, :WO], mr[:, 0:1, :WO])
            v.tensor_copy(mr[:, 2:4, :WO], mr[:, 0:2, :WO])
            v.tensor_copy(mr[:, 4:8, :WO], mr[:, 0:4, :WO])
            v.tensor_copy(mr[:, 8:16, :WO], mr[:, 0:8, :WO])
            mrep.append(mr)

        # ---- gather/select + output ----
        oc = None
        for ci, (s0, L) in enumerate(CHUNKS):
            xt = xts[ci]
            ot = outp.tile([128, SCH, WOP], f32, name="ot", tag="ot")

            def xw(j):
                dh, dw = j // 2, j % 2
                return xt[:, :L, dh, dw::2]

            v.tensor_mul(ot[:, :L, :WO], xw(0), mrep[0][:, :L, :WO])
            for j in (1, 2, 3):
                v.copy_predicated(ot[:, :L, :WO], mrep[j][:, :L, :WO], xw(j))

            if ci == 0:
                # center row: + (1-exact) * box_center
                v.tensor_add(ot[:, 0, :WO], ot[:, 0, :WO], bc_term[:])
                oc = maskp.tile([128, WO], f32, name="oc")
                v.tensor_copy(oc[:], ot[:, 0, :WO])
            if ci == len(CHUNKS) - 1:
                # noise row: abs, + (1-exact) * box_noise
                sc.activation(ot[:, L - 1, :WO], ot[:, L - 1, :WO], Act.Abs)
                v.tensor_add(ot[:, L - 1, :WO], ot[:, L - 1, :WO], bn_term[:])

            for si in range(L):
                dst = y_d[s0 + si, :, hob:hob + HOG, :]
                eng = nc.sync if si % 2 == 1 else nc.scalar
                eng.dma_start(dst, ot[:, si, :WO])

        # ---- x_min / x_max ----
        es = maskp.tile([128, WO], f32, name="es")
        v.tensor_mul(es[:], win(eps_w, 0), ohx[0][:])
        for j in (1, 2, 3):
            v.copy_predicated(es[:], ohxi[j][:], win(eps_w, j))
        oabs = maskp.tile([128, WO], f32, name="oabs")
        v.tensor_add(oabs[:], es[:], bn_term[:])
        xmin_t = maskp.tile([128, WO], f32, name="xmin_t")
        xmax_t = maskp.tile([128, WO], f32, name="xmax_t")
        v.tensor_sub(xmin_t[:], oc[:], oabs[:])
        v.tensor_add(xmax_t[:], oc[:], oabs[:])
        nc.sync.dma_start(ymin_d[:, hob:hob + HOG, :], xmin_t[:])
        nc.sync.dma_start(ymax_d[:, hob:hob + HOG, :], xmax_t[:])


def _build(repeat=1, mode='full'):
    from contextlib import ExitStack
    import concourse.bass as bass
    import concourse.tile as tile
    from concourse import bacc, mybir

    nc = bacc.Bacc("TRN2", target_bir_lowering=False, debug=False,
                   enable_asserts=False, num_devices=NCORES)
    f32 = mybir.dt.float32
    x_d = nc.dram_tensor("x", (S, CL, H, W), f32, kind="ExternalInput").ap()
    xt_d = nc.dram_tensor("x_true", (CL, H, W), f32, kind="ExternalInput").ap()
    y_d = nc.dram_tensor("y", (S, CL, HO, WO), f32, kind="ExternalOutput").ap()
    ymin_d = nc.dram_tensor("y_min", (CL, HO, WO), f32, kind="ExternalOutput").ap()
    ymax_d = nc.dram_tensor("y_max", (CL, HO, WO), f32, kind="ExternalOutput").ap()
    ytrue_d = nc.dram_tensor("y_true", (CL, HO, WO), f32, kind="ExternalOutput").ap()

    with tile.TileContext(nc) as tc:
        from contextlib import ExitStack as _ES
        for _ in range(repeat):
            with _ES() as ctx:
                _emit(ctx, tc, nc, mybir, bass, x_d, xt_d, y_d, ymin_d, ymax_d,
                      ytrue_d, mode=mode)
    nc.compile()
    return nc


def _get_nc():
    if "nc" not in _CACHE:
        _CACHE["nc"] = _build()
    return _CACHE["nc"]


def kernel(x, x_true, kernel_size, stride, trace=False):
    from concourse.bass_utils import run_bass_kernel_spmd

    x = np.asarray(x, dtype=np.float32)
    x_true = np.asarray(x_true, dtype=np.float32)
    assert int(kernel_size) == 2 and int(stride) == 2
    assert x.shape == (S, C, H, W) and x_true.shape == (C, H, W)

    nc = _get_nc()
    in_maps = [
        {
            "x": np.ascontiguousarray(x[:, m * CL:(m + 1) * CL]),
            "x_true": np.ascontiguousarray(x_true[m * CL:(m + 1) * CL]),
        }
        for m in range(NCORES)
    ]
    res = run_bass_kernel_spmd(nc, in_maps, core_ids=list(range(NCORES)),
                               trace=trace)
    if trace:
        _CACHE["last_results"] = res
    x_out = np.concatenate([r["y"] for r in res.results], axis=1)
    x_min = np.concatenate([r["y_min"] for r in res.results], axis=0)
    x_max = np.concatenate([r["y_max"] for r in res.results], axis=0)
    x_true_out = np.concatenate([r["y_true"] for r in res.results], axis=0)
    return x_out, x_min, x_max, x_true_out
